# revision 1
# baseline (speedup 1.0000x reference)
"""GraphConv VAE encoder (3x GraphConv + reparameterization) on 8 Trainium2 cores.

Strategy (graph/data parallel, dst-sharded):
  - Nodes padded to NPAD = 8*SH and sharded by dst across 8 cores.
  - Layer-1 projection hp = (feat @ W1) * ns computed on each core for its own
    node shard (host pre-transposes feat so no on-chip transposes are needed),
    then AllGather -> full bf16 gather table.
  - Edges are dst-sorted into 128-dst "sblocks", grouped 8 sblocks per
    supergroup, and split into 4 src-range buckets (dma_gather indices are
    int16, so gather tables are addressed in 4 windows of NPAD/4 rows).
  - Per 128-edge chunk: dma_gather the source rows (partition = edge), build a
    one-hot selection matrix S via iota==dstloc on DVE, and matmul S^T @ rows
    into a per-sblock PSUM accumulator.  Segment-sum therefore runs on the
    tensor engine; each sblock is one PSUM->SBUF copy, no read-modify-write.
  - h = relu(agg*nd + b1) * ns is transposed per-tile on the PE and
    AllGathered feature-major, so the replicated layer-2/3 projection
    hp23 = h @ [W_mu | W_ls] needs no transposes and lands row-major for the
    second gather pass.
  - Final epilogue mu + noise * exp(log_sigma) is fused per sblock.
"""

import sys

sys.path.insert(0, '/opt/trn_rl_repo')

import numpy as np
import ml_dtypes

import concourse.bass as bass
import concourse.bacc as bacc
import concourse.mybir as mybir
import concourse.tile as tile
from concourse import library_config
from concourse.tile_rust import add_dep_helper
from concourse.vector_clock import ScopedClock
from concourse.bass_utils import run_bass_kernel_spmd

BF16 = mybir.dt.bfloat16
F32 = mybir.dt.float32
NPBF16 = ml_dtypes.bfloat16

NC = 8          # cores
P = 128         # partitions / sblock width
SG = 8          # sblocks per supergroup (one PSUM bank per sblock)
NBUCK = 4       # src-range buckets (int16 gather index limit)
PAD_DSTLOC = 256.0  # dstloc value for padded slots (never matches iota 0..127)


def _patch_tile_drain():
    """This walrus build rejects >1 sync-wait on the kernel-tail Drain; spread
    the waits across chained drains."""
    if getattr(tile.TileContext, "_drain_patched", False):
        return

    def patched(self, tick_clock, wait_clock):
        drain_inst = self.nc.sync.drain()
        wait_clock.add_sem_waits(drain_inst.ins,
                                 ScopedClock({None: tick_clock.global_clock}))
        si = drain_inst.ins.sync_info
        if si is not None and si.on_wait and len(si.on_wait) > 1:
            waits = list(si.on_wait)
            si.on_wait = waits[:1]
            for w in waits[1:]:
                d2 = self.nc.sync.drain()
                d2.ins.sync_info = mybir.SyncInfo(on_wait=[w], on_update=[])
        self.nc.all_engine_barrier()
        assert self.sems is not None
        popped = self.nc._tile_sem_poison_stack.pop()
        assert popped is self._sem_poison
        self.nc.clear_and_free_semaphores(list(self.sems.allocated().values()))
        self.nc.all_engine_barrier()

    tile.TileContext._drain_and_barrier = patched
    tile.TileContext._drain_patched = True


def _build_template(edges, n_nodes, npad):
    """Host-side edge preprocessing shared by both gather passes.

    Returns the SPMD-shared template (chunk counts / call table / chunk
    metadata) and the per-core slot data (int16 gather indices, dstloc).
    """
    src = edges[0].astype(np.int64)
    dst = edges[1].astype(np.int64)
    sh = npad // NC          # nodes per core shard
    nsb = sh // P            # sblocks per core
    brows = npad // NBUCK    # rows per gather bucket
    n_sg = (nsb + SG - 1) // SG
    sgs = [list(range(g * SG, min((g + 1) * SG, nsb))) for g in range(n_sg)]

    core = dst // sh
    k = (dst % sh) // P
    b = src // brows
    # cell id: (core, sg, b, k) major->minor defines the stream order
    sg_of_k = k // SG
    cell = ((core * n_sg + sg_of_k) * NBUCK + b) * nsb + k
    n_cells = NC * n_sg * NBUCK * nsb
    cnt = np.bincount(cell, minlength=n_cells).reshape(NC, n_sg, NBUCK, nsb)

    # shared chunk counts per (k, b): max over cores, >=1 chunk
    # C[k, b] indexed by absolute k
    C = np.zeros((nsb, NBUCK), np.int64)
    for g, ks in enumerate(sgs):
        for kk in ks:
            for bb in range(NBUCK):
                mx = cnt[:, g, bb, kk].max()
                C[kk, bb] = max(1, -(-int(mx) // P))

    # slot offsets in template order: for g: for b: for k in sgs[g]
    cell_order = []          # (g, b, k) in stream order
    for g, ks in enumerate(sgs):
        for bb in range(NBUCK):
            for kk in ks:
                cell_order.append((g, bb, kk))
    cell_slots = np.array([C[kk, bb] * P for (_, bb, kk) in cell_order])
    cell_off = np.concatenate([[0], np.cumsum(cell_slots)[:-1]])
    total_slots = int(cell_slots.sum())
    n_chunks = total_slots // P

    # call table: one dma_gather per (g, b)
    calls = []               # (g, b, slot_off, num_idxs)
    pos = 0
    idx_in_order = {}
    for i, (g, bb, kk) in enumerate(cell_order):
        idx_in_order[(g, bb, kk)] = i
    for g, ks in enumerate(sgs):
        for bb in range(NBUCK):
            ni = int(sum(C[kk, bb] for kk in ks)) * P
            calls.append((g, bb, pos, ni))
            pos += ni
    assert pos == total_slots

    # chunk metadata in stream order: (k, g, b, start, stop)
    chunks = []
    for (g, bb, kk) in cell_order:
        nch = int(C[kk, bb])
        for j in range(nch):
            start = (bb == 0 and j == 0)
            stop = (bb == NBUCK - 1 and j == nch - 1)
            chunks.append((kk, g, bb, start, stop))
    assert len(chunks) == n_chunks

    # per-core slot data
    order = np.argsort(cell, kind='stable')
    cell_sorted = cell[order]
    # rank within cell
    cell_start = np.searchsorted(cell_sorted, np.arange(n_cells), side='left')
    rank = np.arange(len(order)) - cell_start[cell_sorted]
    # map cell -> slot offset (per its core's template)
    cell_to_off = np.zeros(n_cells, np.int64)
    for ci, (g, bb, kk) in enumerate(cell_order):
        for c in range(NC):
            gcell = ((c * n_sg + g) * NBUCK + bb) * nsb + kk
            cell_to_off[gcell] = cell_off[ci]
    slot = cell_to_off[cell_sorted] + rank

    idx_vals = np.zeros((NC, total_slots), np.int16)
    dl_vals = np.full((NC, total_slots), PAD_DSTLOC, np.float32)
    csrc = src[order] - b[order] * brows
    cdst = dst[order] % P
    ccore = core[order]
    idx_vals[ccore, slot] = csrc.astype(np.int16)
    dl_vals[ccore, slot] = cdst.astype(np.float32)

    # wrap indices per call: within a call, slot j -> [j%16, off//16 + j//16]
    ni16 = total_slots // 16
    idx16 = np.zeros((NC, 16, ni16), np.int16)
    for (_, _, off, ni) in calls:
        blk = idx_vals[:, off:off + ni].reshape(NC, ni // 16, 16)
        idx16[:, :, off // 16:(off + ni) // 16] = blk.transpose(0, 2, 1)
    idx16 = np.tile(idx16, (1, 8, 1))  # replicate to 128 partitions

    # dstloc per chunk column: [p, ch] = dstloc of slot ch*128+p
    dstloc = dl_vals.reshape(NC, n_chunks, P).transpose(0, 2, 1)  # [NC,128,NCH]
    dstloc = dstloc.astype(NPBF16)

    tpl = dict(sh=sh, nsb=nsb, brows=brows, sgs=sgs, calls=calls,
               chunks=chunks, n_chunks=n_chunks, total_slots=total_slots,
               ni16=ni16)
    return tpl, idx16, dstloc


def _build(feat, edges, W1, b1, W_mu, b_mu, W_ls, b_ls, noise):
    import os
    skip = os.environ.get("K_SKIP", "")
    repeat = int(os.environ.get("K_REPEAT", "1"))
    N, IN = feat.shape
    OUT = W1.shape[1]
    F2 = 2 * OUT
    assert OUT == P
    npad = -(-N // (NC * P)) * NC * P        # multiple of 8*128
    # bucket rows must fit int16 and divide into 128-aligned shards
    while npad % (NBUCK * P) != 0:
        npad += NC * P
    sh = npad // NC
    brows = npad // NBUCK
    assert brows <= 32768
    nsb = sh // P
    kin = IN // P

    tpl, idx16, dstloc = _build_template(edges, N, npad)
    sgs, calls, chunks = tpl['sgs'], tpl['calls'], tpl['chunks']
    n_chunks, ni16 = tpl['n_chunks'], tpl['ni16']

    # ---- host-side numeric prep (degrees from the index arrays) ----
    deg_out = np.bincount(edges[0], minlength=npad).astype(np.float64)
    deg_in = np.bincount(edges[1], minlength=npad).astype(np.float64)
    ns = np.clip(deg_out, 1.0, None) ** -0.5
    nd = np.clip(deg_in, 1.0, None) ** -0.5
    ns[N:] = 0.0
    nd[N:] = 0.0
    ns = ns.astype(np.float32)
    nd = nd.astype(np.float32)

    featp = np.zeros((npad, IN), np.float32)
    featp[:N] = feat
    noisep = np.zeros((npad, OUT), np.float32)
    noisep[:N] = noise

    featb = featp.astype(NPBF16)
    W1b = np.ascontiguousarray(W1.astype(NPBF16))
    W23 = np.concatenate([W_mu, W_ls], axis=1)
    W23b = np.ascontiguousarray(W23.astype(NPBF16))
    # W1 as [128, kin, 128]: [p, kc, j] = W1[kc*128+p, j]
    W1sb = np.ascontiguousarray(W1b.reshape(kin, P, OUT).transpose(1, 0, 2))

    iota4 = np.tile(np.arange(P, dtype=np.float32), 8)[None, :].repeat(P, 0)
    iota4 = iota4.astype(NPBF16)                       # [128, 1024]
    ident = np.eye(P, dtype=np.float32).astype(NPBF16)  # [128, 128]
    b1r = np.tile(b1[None, :].astype(np.float32), (P, 1))
    bmur = np.tile(b_mu[None, :].astype(np.float32), (P, 1))
    blsr = np.tile(b_ls[None, :].astype(np.float32), (P, 1))

    in_maps = []
    for c in range(NC):
        rows = slice(c * sh, (c + 1) * sh)
        fsh = featb[rows]                               # [sh, IN]
        featT = np.ascontiguousarray(
            fsh.T.reshape(kin, P, sh).transpose(1, 0, 2).reshape(P, kin * sh))
        nsc = np.ascontiguousarray(
            ns[rows].reshape(nsb, P).T)                 # [128, nsb]
        ndc = np.ascontiguousarray(nd[rows].reshape(nsb, P).T)
        noc = np.ascontiguousarray(
            noisep[rows].reshape(nsb, P, OUT).transpose(1, 0, 2)
            .reshape(P, nsb * OUT))                     # [128, nsb*128]
        in_maps.append({
            "featT": featT, "W1sb": W1sb.reshape(P, kin * OUT),
            "W23sb": W23b, "b1r": b1r, "bmur": bmur, "blsr": blsr,
            "nsc": nsc, "ndc": ndc, "noise_sb": noc,
            "iota4": iota4, "ident": ident,
            "idx16": np.ascontiguousarray(idx16[c]),
            "dstloc": np.ascontiguousarray(dstloc[c]),
        })

    # ---------------- device program ----------------
    _patch_tile_drain()
    nc = bacc.Bacc('TRN2', target_bir_lowering=False, debug=False)

    featT_d = nc.dram_tensor("featT", [P, kin * sh], BF16, kind="ExternalInput")
    W1_d = nc.dram_tensor("W1sb", [P, kin * OUT], BF16, kind="ExternalInput")
    W23_d = nc.dram_tensor("W23sb", [P, F2], BF16, kind="ExternalInput")
    b1_d = nc.dram_tensor("b1r", [P, OUT], F32, kind="ExternalInput")
    bmu_d = nc.dram_tensor("bmur", [P, OUT], F32, kind="ExternalInput")
    bls_d = nc.dram_tensor("blsr", [P, OUT], F32, kind="ExternalInput")
    ns_d = nc.dram_tensor("nsc", [P, nsb], F32, kind="ExternalInput")
    nd_d = nc.dram_tensor("ndc", [P, nsb], F32, kind="ExternalInput")
    noise_d = nc.dram_tensor("noise_sb", [P, nsb * OUT], F32,
                             kind="ExternalInput")
    iota_d = nc.dram_tensor("iota4", [P, 8 * P], BF16, kind="ExternalInput")
    ident_d = nc.dram_tensor("ident", [P, P], BF16, kind="ExternalInput")
    idx_d = nc.dram_tensor("idx16", [P, ni16], mybir.dt.int16,
                           kind="ExternalInput")
    dl_d = nc.dram_tensor("dstloc", [P, n_chunks], BF16, kind="ExternalInput")
    y_d = nc.dram_tensor("y", [sh, OUT], F32, kind="ExternalOutput")

    replica = [list(range(NC))]

    with tile.TileContext(nc) as tc:
        import contextlib
        with contextlib.ExitStack() as ctx:
            dram = ctx.enter_context(tc.tile_pool(name="dram", bufs=1,
                                                  space="DRAM"))
            cpool = ctx.enter_context(tc.tile_pool(name="const", bufs=1))
            psum = ctx.enter_context(tc.tile_pool(name="psum", bufs=SG,
                                                  space="PSUM"))

            hp_bounce = dram.tile([sh, OUT], BF16, tag="hp_bounce")
            ka = (len(sgs) // 2) * SG if nsb >= 2 * SG else nsb
            kb = nsb - ka
            hT_bounce = dram.tile([P, ka * P], BF16, tag="hT_bounce")
            hT_bounce_b = (dram.tile([P, kb * P], BF16, tag="hT_bounce_b",
                                      name="hT_bounce_b")
                           if kb else None)
            hp_fulls = [dram.tile([npad, OUT], BF16, tag=f"hp_full_{r}",
                                  addr_space="Shared", name=f"hp_full_{r}")
                        for r in range(repeat)]
            hT_fulls = [dram.tile([NC * P, ka * P], BF16, tag=f"hT_full_{r}",
                                  addr_space="Shared", name=f"hT_full_{r}")
                        for r in range(repeat)]
            hT_fulls_b = [dram.tile([NC * P, kb * P], BF16,
                                    tag=f"hT_fullb_{r}",
                                    addr_space="Shared",
                                    name=f"hT_fullb_{r}")
                          for r in range(repeat)] if kb else None
            hp23_b = [dram.tile([brows, F2], BF16, tag=f"hp23_{b}",
                                name=f"hp23_{b}")
                      for b in range(NBUCK)]

            # constants
            W1_t = cpool.tile([P, kin, OUT], BF16, tag="w1")
            W23_t = cpool.tile([P, F2], BF16, tag="w23")
            b1_t = cpool.tile([P, OUT], F32, tag="b1")
            bmu_t = cpool.tile([P, OUT], F32, tag="bmu")
            bls_t = cpool.tile([P, OUT], F32, tag="bls")
            ns_t = cpool.tile([P, nsb], F32, tag="ns")
            nd_t = cpool.tile([P, nsb], F32, tag="nd")
            iota_t = cpool.tile([P, 8, P], BF16, tag="iota")
            ident_t = cpool.tile([P, P], BF16, tag="ident")
            idx_t = cpool.tile([P, ni16], mybir.dt.int16, tag="idx")
            dl_t = cpool.tile([P, n_chunks], BF16, tag="dl")
            nc.sync.dma_start(out=W1_t[:], in_=W1_d[:].rearrange(
                "p (k o) -> p k o", k=kin))
            nc.sync.dma_start(out=W23_t[:], in_=W23_d[:])
            nc.sync.dma_start(out=b1_t[:], in_=b1_d[:])
            nc.sync.dma_start(out=bmu_t[:], in_=bmu_d[:])
            nc.sync.dma_start(out=bls_t[:], in_=bls_d[:])
            nc.sync.dma_start(out=ns_t[:], in_=ns_d[:])
            nc.sync.dma_start(out=nd_t[:], in_=nd_d[:])
            nc.sync.dma_start(out=iota_t[:], in_=iota_d[:].rearrange(
                "p (a b) -> p a b", a=8))
            nc.sync.dma_start(out=ident_t[:], in_=ident_d[:])
            nc.sync.dma_start(out=idx_t[:], in_=idx_d[:])
            nc.sync.dma_start(out=dl_t[:], in_=dl_d[:])

            reload_inst = nc.gpsimd.load_library(library_config.mlp)

            _cpk = {}
            for (k2, g2, b2, st2, sp2) in chunks:
                _cpk[(k2, b2)] = _cpk.get((k2, b2), 0) + 1

            max_call_chunks = max(ni // P for (_, _, _, ni) in calls)

            def gather_pass(table_aps, elem, gpool, spool, chunk_sink):
                """Shared structure of P2/P4: per-(sg,b) dma_gather calls, S
                build per 4 chunks, matmul per chunk into per-sblock psums.
                chunk_sink(k_abs, ps) is called when a sblock finishes."""
                ps_of = {}
                s4 = None
                ch = 0
                ci = 0
                for g, ks in enumerate(sgs):
                    for kk in ks:
                        ps_of[kk] = psum.tile([P, elem], F32, tag="acc",
                                              name=f"acc_{kk}")
                    for bb in range(NBUCK):
                        (gg, bb2, off, nidx) = calls[ci]
                        assert gg == g and bb2 == bb
                        ci += 1
                        gt = gpool.tile([P, max_call_chunks, elem], BF16,
                                        tag="gt")
                        gi = nc.gpsimd.dma_gather(
                            out_ap=gt[:, :nidx // P, :],
                            in_ap=table_aps[bb],
                            idxs_ap=idx_t[:, off // 16:(off + nidx) // 16],
                            num_idxs=nidx, num_idxs_reg=nidx,
                            elem_size=elem, single_packet=False)
                        add_dep_helper(gi.ins, reload_inst.ins, sync=False)
                        local = 0
                        for kk in ks:
                            nchk = _cpk[(kk, bb)]
                            for j in range(nchk):
                                if ch % 8 == 0:
                                    s4 = spool.tile([P, 8, P], BF16, tag="s4")
                                    n4 = min(8, n_chunks - ch)
                                    nc.vector.tensor_tensor(
                                        out=s4[:, :n4, :],
                                        in0=iota_t[:, :n4, :],
                                        in1=dl_t[:, ch:ch + n4, None]
                                        .to_broadcast([P, n4, P]),
                                        op=mybir.AluOpType.is_equal)
                                kk_, g_, bb_, st, sp = chunks[ch]
                                assert kk_ == kk and g_ == g and bb_ == bb
                                nc.tensor.matmul(
                                    ps_of[kk][:], lhsT=s4[:, ch % 8, :],
                                    rhs=gt[:, local, :], start=st, stop=sp)
                                ch += 1
                                local += 1
                    for kk in ks:
                        chunk_sink(kk, ps_of[kk])
                assert ch == n_chunks

            def one_iter(hp_full, hT_full, hT_full_b):
                # ------------- P1: hp = (feat @ W1) * ns -------------
                with tc.tile_pool(name="featT", bufs=1) as fpool, \
                     tc.tile_pool(name="p1work", bufs=4) as wpool:
                    fT = fpool.tile([P, kin, sh], BF16, tag="fT", name="fT")
                    STRIP = 8
                    for s0 in range(0, nsb, STRIP):
                        s1 = min(s0 + STRIP, nsb)
                        # load only this strip's feature columns so the first
                        # matmuls (and thus AG1) start ~30us earlier
                        nc.sync.dma_start(
                            out=fT[:, :, s0 * P:s1 * P],
                            in_=featT_d[:].rearrange(
                                "p (k s) -> p k s", k=kin)[:, :, s0 * P:s1 * P])
                        strip = wpool.tile([P, STRIP, OUT], BF16,
                                           tag="hpstrip", name="hpstrip")
                        for rt in range(s0, s1):
                            ps = psum.tile([P, OUT], F32, tag="acc",
                                           name="p1ps")
                            for kc in range(kin):
                                nc.tensor.matmul(
                                    ps[:],
                                    lhsT=fT[:, kc, rt * P:(rt + 1) * P],
                                    rhs=W1_t[:, kc, :],
                                    start=(kc == 0), stop=(kc == kin - 1))
                            nc.vector.tensor_scalar_mul(
                                strip[:, rt - s0, :], ps[:],
                                ns_t[:, rt:rt + 1])
                        nc.sync.dma_start(
                            out=hp_bounce[:].rearrange("(t p) o -> p t o",
                                                       p=P)[:, s0:s1, :],
                            in_=strip[:, :s1 - s0, :])

                if "ag" not in skip:
                    nc.gpsimd.collective_compute(
                        "AllGather", mybir.AluOpType.bypass,
                        ins=[hp_bounce.opt()], outs=[hp_full.opt()],
                        replica_groups=replica)

                # ------------- P2: gather+aggregate layer 1 -> hT ------
                with tc.tile_pool(name="g1", bufs=4) as gpool, \
                     tc.tile_pool(name="s1", bufs=4) as spool, \
                     tc.tile_pool(name="h1", bufs=4) as hpool, \
                     tc.tile_pool(name="hts", bufs=2) as htspool:

                    ht_strips = {}

                    def sink1(kk, ps):
                        g = kk // SG
                        j = kk % SG
                        if j == 0:
                            ht_strips[g] = htspool.tile(
                                [P, SG, P], BF16, tag="hts", name=f"hts_{g}")
                        t1 = hpool.tile([P, OUT], F32, tag="t1", name="t1")
                        nc.vector.tensor_scalar_mul(t1[:], ps[:],
                                                    nd_t[:, kk:kk + 1])
                        nc.vector.tensor_tensor(out=t1[:], in0=t1[:],
                                                in1=b1_t[:],
                                                op=mybir.AluOpType.add)
                        hrow = hpool.tile([P, OUT], BF16, tag="hrow",
                                          name="hrow")
                        nc.scalar.activation(
                            hrow[:], t1[:],
                            mybir.ActivationFunctionType.Relu)
                        hs = hpool.tile([P, OUT], BF16, tag="hs", name="hs")
                        nc.vector.tensor_scalar_mul(hs[:], hrow[:],
                                                    ns_t[:, kk:kk + 1])
                        psT = psum.tile([P, P], BF16, tag="acc", name="psT")
                        nc.tensor.transpose(psT[:], hs[:], ident_t[:])
                        nc.scalar.activation(
                            ht_strips[g][:, j, :], psT[:],
                            mybir.ActivationFunctionType.Copy)
                        last = (kk == nsb - 1)
                        if j == SG - 1 or last:
                            n = j + 1
                            k0 = kk - j
                            if k0 >= ka:
                                nc.sync.dma_start(
                                    out=hT_bounce_b[:, (k0 - ka) * P:
                                                    (kk + 1 - ka) * P],
                                    in_=ht_strips[g][:, :n, :].rearrange(
                                        "p a b -> p (a b)"))
                            else:
                                nc.sync.dma_start(
                                    out=hT_bounce[:, k0 * P:(kk + 1) * P],
                                    in_=ht_strips[g][:, :n, :].rearrange(
                                        "p a b -> p (a b)"))

                    if "gather" not in skip:
                        gather_pass([hp_full[bb * brows:(bb + 1) * brows, :]
                                     for bb in range(NBUCK)], OUT, gpool,
                                    spool, sink1)

                if "ag" not in skip:
                    nc.gpsimd.collective_compute(
                        "AllGather", mybir.AluOpType.bypass,
                        ins=[hT_bounce.opt()], outs=[hT_full.opt()],
                        replica_groups=replica)
                    if kb:
                        nc.gpsimd.collective_compute(
                            "AllGather", mybir.AluOpType.bypass,
                            ins=[hT_bounce_b.opt()], outs=[hT_full_b.opt()],
                            replica_groups=replica)

                # ------------- P3: hp23 = h @ [Wmu|Wls] (replicated) ----
                with tc.tile_pool(name="p3in", bufs=3) as ipool, \
                     tc.tile_pool(name="p3out", bufs=3) as opool:
                    LD = 8
                    nt_b = brows // P
                    STO = next(d for d in (8, 7, 6, 5, 4, 3, 2, 1)
                               if nt_b % d == 0)
                    rt = 0
                    ost = None
                    n_tiles = npad // P
                    while rt < n_tiles:
                        c = (rt * P) // sh
                        kloc = (rt - c * (sh // P))
                        lim = (ka - kloc) if kloc < ka else (nsb - kloc)
                        nld = min(LD, lim, n_tiles - rt)
                        lt = ipool.tile([P, LD, P], BF16, tag="ld", name="ld")
                        if kloc < ka:
                            src_ht = hT_full[c * P:(c + 1) * P,
                                             kloc * P:(kloc + nld) * P]
                        else:
                            src_ht = hT_full_b[c * P:(c + 1) * P,
                                               (kloc - ka) * P:
                                               (kloc - ka + nld) * P]
                        nc.sync.dma_start(
                            out=lt[:, :nld, :],
                            in_=src_ht.rearrange("p (a b) -> p a b", b=P))
                        for j in range(nld):
                            rtj = rt + j
                            if (rtj % STO) == 0:
                                ost = opool.tile([P, STO, F2], BF16, tag="st",
                                                 name="st")
                            ps23 = psum.tile([P, F2], F32, tag="acc",
                                             name="ps23")
                            nc.tensor.matmul(ps23[:], lhsT=lt[:, j, :],
                                             rhs=W23_t[:], start=True,
                                             stop=True)
                            if rtj % 2 == 0:
                                nc.vector.tensor_copy(ost[:, rtj % STO, :],
                                                      ps23[:])
                            else:
                                nc.scalar.activation(
                                    ost[:, rtj % STO, :], ps23[:],
                                    mybir.ActivationFunctionType.Copy)
                            if (rtj % STO) == STO - 1:
                                bb = rtj // nt_b
                                t0 = (rtj - (STO - 1)) % nt_b
                                nc.sync.dma_start(
                                    out=hp23_b[bb][:].rearrange(
                                        "(t p) f -> p t f", p=P)
                                    [:, t0:t0 + STO, :],
                                    in_=ost[:])
                        rt += nld

                # ------------- P4: gather+aggregate layers 2/3 ----------
                with tc.tile_pool(name="g2", bufs=3) as gpool2, \
                     tc.tile_pool(name="s2", bufs=4) as spool2, \
                     tc.tile_pool(name="e2", bufs=6) as epool, \
                     tc.tile_pool(name="noisep", bufs=1) as npool, \
                     tc.tile_pool(name="outs", bufs=2) as outpool:

                    noise_t = npool.tile([P, nsb, OUT], F32, tag="noise",
                                         name="noise")
                    nc.sync.dma_start(out=noise_t[:],
                                      in_=noise_d[:].rearrange(
                                          "p (k o) -> p k o", k=nsb))

                    out_strips = {}

                    def sink2(kk, ps):
                        g = kk // SG
                        j = kk % SG
                        if j == 0:
                            out_strips[g] = outpool.tile(
                                [P, SG, OUT], F32, tag="outs",
                                name=f"os_{g}")
                        tmu = epool.tile([P, OUT], F32, tag="tmu", name="tmu")
                        nc.vector.tensor_scalar_mul(tmu[:], ps[:, 0:OUT],
                                                    nd_t[:, kk:kk + 1])
                        nc.vector.tensor_tensor(out=tmu[:], in0=tmu[:],
                                                in1=bmu_t[:],
                                                op=mybir.AluOpType.add)
                        tls = epool.tile([P, OUT], F32, tag="tls", name="tls")
                        nc.vector.tensor_scalar_mul(tls[:], ps[:, OUT:F2],
                                                    nd_t[:, kk:kk + 1])
                        nc.vector.tensor_tensor(out=tls[:], in0=tls[:],
                                                in1=bls_t[:],
                                                op=mybir.AluOpType.add)
                        sig = epool.tile([P, OUT], F32, tag="sig", name="sig")
                        nc.scalar.activation(
                            sig[:], tls[:],
                            mybir.ActivationFunctionType.Exp)
                        nc.vector.tensor_tensor(out=sig[:], in0=sig[:],
                                                in1=noise_t[:, kk, :],
                                                op=mybir.AluOpType.mult)
                        nc.vector.tensor_tensor(out=out_strips[g][:, j, :],
                                                in0=tmu[:], in1=sig[:],
                                                op=mybir.AluOpType.add)
                        last = (kk == nsb - 1)
                        if j == SG - 1 or last:
                            n = j + 1
                            k0 = kk - j
                            nc.sync.dma_start(
                                out=y_d[:].rearrange("(t p) o -> p t o",
                                                     p=P)[:, k0:k0 + n, :],
                                in_=out_strips[g][:, :n, :])

                    if "gather" not in skip:
                        gather_pass([hp23_b[bb][:] for bb in range(NBUCK)],
                                    F2, gpool2, spool2, sink2)

            for _rep in range(repeat):
                one_iter(hp_fulls[_rep], hT_fulls[_rep],
                         hT_fulls_b[_rep] if kb else None)

    nc.compile()
    return nc, in_maps, N


_CACHE = {}


def _run(feat, edges, W1, b1, W_mu, b_mu, W_ls, b_ls, noise, trace=False):
    import hashlib
    h = hashlib.sha1()
    for a in (edges, feat, W1, b1, W_mu, b_mu, W_ls, b_ls, noise):
        h.update(np.ascontiguousarray(a).tobytes())
    key = h.hexdigest()
    if key in _CACHE:
        nc, in_maps, N = _CACHE[key]
    else:
        nc, in_maps, N = _build(feat, edges, W1, b1, W_mu, b_mu, W_ls, b_ls,
                                noise)
        _CACHE[key] = (nc, in_maps, N)
    res = run_bass_kernel_spmd(nc, in_maps, core_ids=list(range(NC)),
                               trace=trace)
    out = np.concatenate([res.results[c]["y"] for c in range(NC)], axis=0)
    return out[:N], res


def kernel(feat, edges, W1, b1, W_mu, b_mu, W_ls, b_ls, noise):
    out, _ = _run(np.asarray(feat), np.asarray(edges), np.asarray(W1),
                  np.asarray(b1), np.asarray(W_mu), np.asarray(b_mu),
                  np.asarray(W_ls), np.asarray(b_ls), np.asarray(noise))
    return out



# revision 4
# speedup vs baseline: 1.2522x; 1.2522x over previous
"""GraphConv VAE encoder (3x GraphConv + reparameterization) on 8 Trainium2 cores.

Strategy (graph/data parallel, dst-sharded, aggregate-then-project):
  - Nodes padded to NPAD = 8*SH and sharded by dst across 8 cores.
  - P1: hp = (feat @ W1) * ns computed per-core on its own node shard (host
    pre-transposes feat so no on-chip transposes are needed), then
    AllGather -> full bf16 gather table hp_full [npad, 128].
  - Edges are dst-sorted into 128-dst "sblocks", grouped 8 sblocks per
    supergroup, and split into 4 src-range buckets (dma_gather indices are
    int16, so gather tables are addressed in 4 windows of NPAD/4 rows).
  - P2: per 128-edge chunk, dma_gather source rows (partition = edge), build a
    one-hot selection matrix S via iota==dstloc on DVE, matmul S^T @ rows into
    a per-sblock PSUM accumulator (segment-sum on the tensor engine).  Sink:
    h = relu(agg*nd + b1); hs = h*ns stored ROW-major -> AllGather ->
    hs_full [npad, 128].
  - P4: same gather structure over hs_full, but accumulated TRANSPOSED:
    aggT = rows^T @ S ([feat, dst] in PSUM).  Since segment-sum commutes with
    the right-weight matmul, project only the local shard afterwards:
    ps23 = aggT^T @ [W_mu | W_ls] directly (aggT is already the lhsT).
    Epilogue mu + noise * exp(log_sigma) fused per sblock.
  - No replicated layer-2/3 projection over all nodes, no transposes, one
    AllGather per layer boundary.
  - The gather passes are descriptor-throughput-bound (~8.8ns/descriptor
    through the SWDGE path), so nodes are REBALANCED host-side: a
    quarter-preserving permutation assigns nodes to (core, sblock) so every
    (sblock, src-bucket) cell holds <= 512 edges = exactly 4 chunks of 128
    (npad is grown so cells average < 500, giving the balancer slack).
    This cuts the shared chunk template from 1953 to 1600 chunks/core/pass.
"""

import sys

sys.path.insert(0, '/opt/trn_rl_repo')

import numpy as np
import ml_dtypes

import concourse.bass as bass
import concourse.bacc as bacc
import concourse.mybir as mybir
import concourse.tile as tile
from concourse import library_config
from concourse.tile_rust import add_dep_helper
from concourse.vector_clock import ScopedClock
from concourse.bass_utils import run_bass_kernel_spmd

BF16 = mybir.dt.bfloat16
F32 = mybir.dt.float32
NPBF16 = ml_dtypes.bfloat16

NC = 8          # cores
P = 128         # partitions / sblock width
SG = 8          # sblocks per supergroup (one PSUM bank per sblock)
NBUCK = 4       # src-range buckets (int16 gather index limit)
PAD_DSTLOC = 256.0  # dstloc value for padded slots (never matches iota 0..127)


def _patch_tile_drain():
    """This walrus build rejects >1 sync-wait on the kernel-tail Drain; spread
    the waits across chained drains."""
    if getattr(tile.TileContext, "_drain_patched", False):
        return

    def patched(self, tick_clock, wait_clock):
        drain_inst = self.nc.sync.drain()
        wait_clock.add_sem_waits(drain_inst.ins,
                                 ScopedClock({None: tick_clock.global_clock}))
        si = drain_inst.ins.sync_info
        if si is not None and si.on_wait and len(si.on_wait) > 1:
            waits = list(si.on_wait)
            si.on_wait = waits[:1]
            for w in waits[1:]:
                d2 = self.nc.sync.drain()
                d2.ins.sync_info = mybir.SyncInfo(on_wait=[w], on_update=[])
        self.nc.all_engine_barrier()
        assert self.sems is not None
        popped = self.nc._tile_sem_poison_stack.pop()
        assert popped is self._sem_poison
        self.nc.clear_and_free_semaphores(list(self.sems.allocated().values()))
        self.nc.all_engine_barrier()

    tile.TileContext._drain_and_barrier = patched
    tile.TileContext._drain_patched = True


def _balance_perm(edges, n_nodes, npad):
    """Node -> table-row permutation equalizing per-(sblock, src-bucket)
    edge counts under 512 (= 4 chunks of 128).  Quarter-preserving for real
    nodes (so src buckets are stable); pad rows spread evenly as slack."""
    sh = npad // NC
    nsb = sh // P
    brows = npad // NBUCK
    qreal = n_nodes // NBUCK
    src = edges[0].astype(np.int64)
    dst = edges[1].astype(np.int64)
    hb = np.zeros((n_nodes, NBUCK), np.int32)
    np.add.at(hb, (dst, np.minimum(src // qreal, NBUCK - 1)), 1)

    def balance_quarter(hv, nbins):
        n = hv.shape[0]
        per = n // nbins
        deg = hv.sum(1)
        rate = hv.sum() / n / NBUCK
        loads = np.zeros((nbins, NBUCK), np.int32)
        counts = np.zeros(nbins, np.int32)
        assign = np.empty(n, np.int32)
        order = np.argsort(-deg, kind='stable')
        CAP = 4 * P
        for v in order:
            h = hv[v]
            newload = loads + h
            fits = (counts < per) & (newload <= CAP).all(1)
            if fits.any():
                resid = newload - (counts[:, None] + 1) * rate
                score = np.where(fits, resid.max(1), 1 << 30)
                s = int(np.argmin(score))
            else:
                over = np.maximum(newload - CAP, 0).sum(1)
                score = np.where(counts < per,
                                 over.astype(np.int64) * (1 << 20)
                                 + newload.max(1), 1 << 62)
                s = int(np.argmin(score))
            assign[v] = s
            loads[s] += h
            counts[s] += 1
        for _ in range(8):
            over_bins = np.where((loads > CAP).any(1))[0]
            if not len(over_bins):
                break
            moved = 0
            slack_order = np.argsort(loads.max(1))
            for s in over_bins:
                members_s = np.where(assign == s)[0]
                vs = members_s[np.argsort(-hv[members_s].sum(1))]
                done = False
                for v in vs[:32]:
                    hvv = hv[v]
                    cap_s = CAP - loads[s] + hvv
                    if (cap_s < 0).any():
                        continue
                    for t in slack_order:
                        if t == s:
                            continue
                        need_t = loads[t] + hvv - CAP
                        members_t = np.where(assign == t)[0]
                        hwt = hv[members_t]
                        ok = ((hwt <= cap_s).all(1)
                              & (hwt >= need_t).all(1))
                        if ok.any():
                            w = members_t[np.argmax(ok)]
                            loads[s] += hv[w] - hvv
                            loads[t] += hvv - hv[w]
                            assign[v], assign[w] = t, s
                            moved += 1
                            done = True
                            break
                    if done:
                        break
            if not moved:
                break
        return assign, loads

    perm = np.empty(npad, np.int64)
    npadq = brows - qreal
    for q in range(NBUCK):
        real = np.arange(q * qreal, (q + 1) * qreal)
        hv = np.concatenate([hb[real],
                             np.zeros((npadq, NBUCK), np.int32)])
        nbins = 2 * nsb
        assign, loads = balance_quarter(hv, nbins)
        chunkv = -(-loads // P)
        srt = {}
        for half in range(2):
            bins = np.arange(half * nsb, (half + 1) * nsb)
            keys = [tuple(-chunkv[s2]) + (-loads[s2].sum(),)
                    for s2 in bins]
            order2 = sorted(range(nsb), key=lambda i: keys[i])
            for newkk, i in enumerate(order2):
                srt[bins[i]] = newkk
        orderv = np.argsort(assign, kind='stable')
        pos_in_bin = np.arange(len(hv)) - np.searchsorted(
            np.sort(assign), np.arange(nbins))[assign[orderv]]
        s_of = assign[orderv]
        c_of = 2 * q + s_of // nsb
        kk_of = np.array([srt[s2] for s2 in s_of])
        rows = c_of * sh + kk_of * P + pos_in_bin
        ids = np.concatenate(
            [real, n_nodes + np.arange(q * npadq, (q + 1) * npadq)])
        perm[ids[orderv]] = rows
    assert np.array_equal(np.sort(perm), np.arange(npad))
    return perm


def _build_template(edges, n_nodes, npad):
    """Host-side edge preprocessing shared by both gather passes.

    Returns the SPMD-shared template (chunk counts / call table / chunk
    metadata) and the per-core slot data (int16 gather indices, dstloc).
    """
    src = edges[0].astype(np.int64)
    dst = edges[1].astype(np.int64)
    sh = npad // NC          # nodes per core shard
    nsb = sh // P            # sblocks per core
    brows = npad // NBUCK    # rows per gather bucket
    n_sg = (nsb + SG - 1) // SG
    sgs = [list(range(g * SG, min((g + 1) * SG, nsb))) for g in range(n_sg)]

    core = dst // sh
    k = (dst % sh) // P
    b = src // brows
    # cell id: (core, sg, b, k) major->minor defines the stream order
    sg_of_k = k // SG
    cell = ((core * n_sg + sg_of_k) * NBUCK + b) * nsb + k
    n_cells = NC * n_sg * NBUCK * nsb
    cnt = np.bincount(cell, minlength=n_cells).reshape(NC, n_sg, NBUCK, nsb)

    # shared chunk counts per (k, b): max over cores, >=1 chunk
    # C[k, b] indexed by absolute k
    C = np.zeros((nsb, NBUCK), np.int64)
    for g, ks in enumerate(sgs):
        for kk in ks:
            for bb in range(NBUCK):
                mx = cnt[:, g, bb, kk].max()
                C[kk, bb] = max(1, -(-int(mx) // P))

    # slot offsets in template order: for g: for b: for k in sgs[g]
    cell_order = []          # (g, b, k) in stream order
    for g, ks in enumerate(sgs):
        for bb in range(NBUCK):
            for kk in ks:
                cell_order.append((g, bb, kk))
    cell_slots = np.array([C[kk, bb] * P for (_, bb, kk) in cell_order])
    cell_off = np.concatenate([[0], np.cumsum(cell_slots)[:-1]])
    total_slots = int(cell_slots.sum())
    n_chunks = total_slots // P

    # call table: one dma_gather per (g, b)
    calls = []               # (g, b, slot_off, num_idxs)
    pos = 0
    for g, ks in enumerate(sgs):
        for bb in range(NBUCK):
            ni = int(sum(C[kk, bb] for kk in ks)) * P
            calls.append((g, bb, pos, ni))
            pos += ni
    assert pos == total_slots

    # chunk metadata in stream order: (k, g, b, start, stop)
    chunks = []
    for (g, bb, kk) in cell_order:
        nch = int(C[kk, bb])
        for j in range(nch):
            start = (bb == 0 and j == 0)
            stop = (bb == NBUCK - 1 and j == nch - 1)
            chunks.append((kk, g, bb, start, stop))
    assert len(chunks) == n_chunks

    # per-core slot data
    order = np.argsort(cell, kind='stable')
    cell_sorted = cell[order]
    # rank within cell
    cell_start = np.searchsorted(cell_sorted, np.arange(n_cells), side='left')
    rank = np.arange(len(order)) - cell_start[cell_sorted]
    # map cell -> slot offset (per its core's template)
    cell_to_off = np.zeros(n_cells, np.int64)
    for ci, (g, bb, kk) in enumerate(cell_order):
        for c in range(NC):
            gcell = ((c * n_sg + g) * NBUCK + bb) * nsb + kk
            cell_to_off[gcell] = cell_off[ci]
    slot = cell_to_off[cell_sorted] + rank

    idx_vals = np.zeros((NC, total_slots), np.int16)
    dl_vals = np.full((NC, total_slots), PAD_DSTLOC, np.float32)
    csrc = src[order] - b[order] * brows
    cdst = dst[order] % P
    ccore = core[order]
    idx_vals[ccore, slot] = csrc.astype(np.int16)
    dl_vals[ccore, slot] = cdst.astype(np.float32)

    # wrap indices per call: within a call, slot j -> [j%16, off//16 + j//16]
    ni16 = total_slots // 16
    idx16 = np.zeros((NC, 16, ni16), np.int16)
    for (_, _, off, ni) in calls:
        blk = idx_vals[:, off:off + ni].reshape(NC, ni // 16, 16)
        idx16[:, :, off // 16:(off + ni) // 16] = blk.transpose(0, 2, 1)
    idx16 = np.tile(idx16, (1, 8, 1))  # replicate to 128 partitions

    # dstloc per chunk column: [p, ch] = dstloc of slot ch*128+p
    dstloc = dl_vals.reshape(NC, n_chunks, P).transpose(0, 2, 1)  # [NC,128,NCH]
    dstloc = dstloc.astype(NPBF16)

    tpl = dict(sh=sh, nsb=nsb, brows=brows, sgs=sgs, calls=calls,
               chunks=chunks, n_chunks=n_chunks, total_slots=total_slots,
               ni16=ni16)
    return tpl, idx16, dstloc


def _build(feat, edges, W1, b1, W_mu, b_mu, W_ls, b_ls, noise):
    import os
    skip = os.environ.get("K_SKIP", "")
    repeat = int(os.environ.get("K_REPEAT", "1"))
    single_packet = bool(int(os.environ.get("K_SP", "0")))
    N, IN = feat.shape
    E = edges.shape[1]
    OUT = W1.shape[1]
    F2 = 2 * OUT
    assert OUT == P
    npad = -(-N // (NC * P)) * NC * P        # multiple of 8*128
    # bucket rows must fit int16 and divide into 128-aligned shards
    while npad % (NBUCK * P) != 0:
        npad += NC * P
    # extra padding so balanced cells average under 500 edges (<512 = 4
    # chunks), giving the balancer slack in both load and node count
    while E / NC / ((npad // NC // P) * NBUCK) > 500.0:
        npad += NC * P
    sh = npad // NC
    brows = npad // NBUCK
    assert brows <= 32768
    nsb = sh // P
    kin = IN // P

    perm = _balance_perm(edges, N, npad)     # node id -> table row
    inv = np.empty(npad, np.int64)
    inv[perm] = np.arange(npad)
    mask = inv < N                           # real rows
    pedges = perm[edges.astype(np.int64)]

    tpl, idx16, dstloc = _build_template(pedges, N, npad)
    sgs, calls, chunks = tpl['sgs'], tpl['calls'], tpl['chunks']
    n_chunks, ni16 = tpl['n_chunks'], tpl['ni16']

    # ---- host-side numeric prep (degrees on node ids, then permuted) ----
    deg_out = np.bincount(edges[0], minlength=N).astype(np.float64)
    deg_in = np.bincount(edges[1], minlength=N).astype(np.float64)
    ns_n = (np.clip(deg_out, 1.0, None) ** -0.5).astype(np.float32)
    nd_n = (np.clip(deg_in, 1.0, None) ** -0.5).astype(np.float32)
    safe = np.minimum(inv, N - 1)
    ns = np.where(mask, ns_n[safe], 0.0).astype(np.float32)
    nd = np.where(mask, nd_n[safe], 0.0).astype(np.float32)

    featp = np.zeros((npad, IN), np.float32)
    featp[mask] = feat[inv[mask]]
    noisep = np.zeros((npad, OUT), np.float32)
    noisep[mask] = noise[inv[mask]]

    featb = featp.astype(NPBF16)
    W1b = np.ascontiguousarray(W1.astype(NPBF16))
    W23 = np.concatenate([W_mu, W_ls], axis=1)
    W23b = np.ascontiguousarray(W23.astype(NPBF16))
    # W1 as [128, kin, 128]: [p, kc, j] = W1[kc*128+p, j]
    W1sb = np.ascontiguousarray(W1b.reshape(kin, P, OUT).transpose(1, 0, 2))

    iota4 = np.tile(np.arange(P, dtype=np.float32), 8)[None, :].repeat(P, 0)
    iota4 = iota4.astype(NPBF16)                       # [128, 1024]
    b1r = np.tile(b1[None, :].astype(np.float32), (P, 1))
    bmur = np.tile(b_mu[None, :].astype(np.float32), (P, 1))
    blsr = np.tile(b_ls[None, :].astype(np.float32), (P, 1))

    in_maps = []
    for c in range(NC):
        rows = slice(c * sh, (c + 1) * sh)
        fsh = featb[rows]                               # [sh, IN]
        featT = np.ascontiguousarray(
            fsh.T.reshape(kin, P, sh).transpose(1, 0, 2).reshape(P, kin * sh))
        nsc = np.ascontiguousarray(
            ns[rows].reshape(nsb, P).T)                 # [128, nsb]
        ndc = np.ascontiguousarray(nd[rows].reshape(nsb, P).T)
        noc = np.ascontiguousarray(
            noisep[rows].reshape(nsb, P, OUT).transpose(1, 0, 2)
            .reshape(P, nsb * OUT))                     # [128, nsb*128]
        in_maps.append({
            "featT": featT, "W1sb": W1sb.reshape(P, kin * OUT),
            "W23sb": W23b, "b1r": b1r, "bmur": bmur, "blsr": blsr,
            "nsc": nsc, "ndc": ndc, "noise_sb": noc,
            "iota4": iota4,
            "idx16": np.ascontiguousarray(idx16[c]),
            "dstloc": np.ascontiguousarray(dstloc[c]),
        })

    # ---------------- device program ----------------
    _patch_tile_drain()
    nc = bacc.Bacc('TRN2', target_bir_lowering=False, debug=False)

    featT_d = nc.dram_tensor("featT", [P, kin * sh], BF16, kind="ExternalInput")
    W1_d = nc.dram_tensor("W1sb", [P, kin * OUT], BF16, kind="ExternalInput")
    W23_d = nc.dram_tensor("W23sb", [P, F2], BF16, kind="ExternalInput")
    b1_d = nc.dram_tensor("b1r", [P, OUT], F32, kind="ExternalInput")
    bmu_d = nc.dram_tensor("bmur", [P, OUT], F32, kind="ExternalInput")
    bls_d = nc.dram_tensor("blsr", [P, OUT], F32, kind="ExternalInput")
    ns_d = nc.dram_tensor("nsc", [P, nsb], F32, kind="ExternalInput")
    nd_d = nc.dram_tensor("ndc", [P, nsb], F32, kind="ExternalInput")
    noise_d = nc.dram_tensor("noise_sb", [P, nsb * OUT], F32,
                             kind="ExternalInput")
    iota_d = nc.dram_tensor("iota4", [P, 8 * P], BF16, kind="ExternalInput")
    idx_d = nc.dram_tensor("idx16", [P, ni16], mybir.dt.int16,
                           kind="ExternalInput")
    dl_d = nc.dram_tensor("dstloc", [P, n_chunks], BF16, kind="ExternalInput")
    y_d = nc.dram_tensor("y", [sh, OUT], F32, kind="ExternalOutput")

    replica = [list(range(NC))]

    with tile.TileContext(nc) as tc:
        import contextlib
        with contextlib.ExitStack() as ctx:
            dram = ctx.enter_context(tc.tile_pool(name="dram", bufs=1,
                                                  space="DRAM"))
            cpool = ctx.enter_context(tc.tile_pool(name="const", bufs=1))
            psum = ctx.enter_context(tc.tile_pool(name="psum", bufs=SG,
                                                  space="PSUM"))

            hp_bounce = dram.tile([sh, OUT], BF16, tag="hp_bounce")
            hs_bounce = dram.tile([sh, OUT], BF16, tag="hs_bounce",
                                  name="hs_bounce")
            hp_fulls = [dram.tile([npad, OUT], BF16, tag=f"hp_full_{r}",
                                  addr_space="Shared", name=f"hp_full_{r}")
                        for r in range(repeat)]
            hs_fulls = [dram.tile([npad, OUT], BF16, tag=f"hs_full_{r}",
                                  addr_space="Shared", name=f"hs_full_{r}")
                        for r in range(repeat)]

            # constants
            W1_t = cpool.tile([P, kin, OUT], BF16, tag="w1")
            W23_t = cpool.tile([P, F2], BF16, tag="w23")
            b1_t = cpool.tile([P, OUT], F32, tag="b1")
            bmu_t = cpool.tile([P, OUT], F32, tag="bmu")
            bls_t = cpool.tile([P, OUT], F32, tag="bls")
            ns_t = cpool.tile([P, nsb], F32, tag="ns")
            nd_t = cpool.tile([P, nsb], F32, tag="nd")
            iota_t = cpool.tile([P, 8, P], BF16, tag="iota")
            idx_t = cpool.tile([P, ni16], mybir.dt.int16, tag="idx")
            dl_t = cpool.tile([P, n_chunks], BF16, tag="dl")
            nc.sync.dma_start(out=W1_t[:], in_=W1_d[:].rearrange(
                "p (k o) -> p k o", k=kin))
            nc.sync.dma_start(out=W23_t[:], in_=W23_d[:])
            nc.sync.dma_start(out=b1_t[:], in_=b1_d[:])
            nc.sync.dma_start(out=bmu_t[:], in_=bmu_d[:])
            nc.sync.dma_start(out=bls_t[:], in_=bls_d[:])
            nc.sync.dma_start(out=ns_t[:], in_=ns_d[:])
            nc.sync.dma_start(out=nd_t[:], in_=nd_d[:])
            nc.sync.dma_start(out=iota_t[:], in_=iota_d[:].rearrange(
                "p (a b) -> p a b", a=8))
            nc.sync.dma_start(out=idx_t[:], in_=idx_d[:])
            nc.sync.dma_start(out=dl_t[:], in_=dl_d[:])

            reload_inst = nc.gpsimd.load_library(library_config.mlp)

            _cpk = {}
            for (k2, g2, b2, st2, sp2) in chunks:
                _cpk[(k2, b2)] = _cpk.get((k2, b2), 0) + 1

            max_call_chunks = max(ni // P for (_, _, _, ni) in calls)

            def gather_pass(table_aps, gpool, spool, chunk_sink,
                            transposed):
                """Per-(sg,b) dma_gather calls, S build per 8 chunks, matmul
                per chunk into per-sblock psums.  transposed=False:
                ps[dst, feat] += S^T @ rows; transposed=True:
                ps[feat, dst] += rows^T @ S.  chunk_sink(k_abs, ps) is called
                when a sblock finishes."""
                ps_of = {}
                s4 = None
                ch = 0
                ci = 0
                for g, ks in enumerate(sgs):
                    for kk in ks:
                        ps_of[kk] = psum.tile([P, OUT], F32, tag="acc",
                                              name=f"acc_{kk}")
                    for bb in range(NBUCK):
                        (gg, bb2, off, nidx) = calls[ci]
                        assert gg == g and bb2 == bb
                        ci += 1
                        gt = gpool.tile([P, max_call_chunks, OUT], BF16,
                                        tag="gt")
                        if not ("g23" in skip and bb >= 2):
                            gi = nc.gpsimd.dma_gather(
                                out_ap=gt[:, :nidx // P, :],
                                in_ap=table_aps[bb],
                                idxs_ap=idx_t[:, off // 16:(off + nidx) // 16],
                                num_idxs=nidx, num_idxs_reg=nidx,
                                elem_size=OUT, single_packet=single_packet)
                            add_dep_helper(gi.ins, reload_inst.ins,
                                           sync=False)
                        local = 0
                        for kk in ks:
                            nchk = _cpk[(kk, bb)]
                            for j in range(nchk):
                                if ch % 8 == 0 and "sb" not in skip:
                                    s4 = spool.tile([P, 8, P], BF16, tag="s4")
                                    n4 = min(8, n_chunks - ch)
                                    nc.vector.tensor_tensor(
                                        out=s4[:, :n4, :],
                                        in0=iota_t[:, :n4, :],
                                        in1=dl_t[:, ch:ch + n4, None]
                                        .to_broadcast([P, n4, P]),
                                        op=mybir.AluOpType.is_equal)
                                elif s4 is None:
                                    s4 = spool.tile([P, 8, P], BF16, tag="s4")
                                kk_, g_, bb_, st, sp = chunks[ch]
                                assert kk_ == kk and g_ == g and bb_ == bb
                                if "mm" not in skip:
                                    if transposed:
                                        nc.tensor.matmul(
                                            ps_of[kk][:], lhsT=gt[:, local, :],
                                            rhs=s4[:, ch % 8, :],
                                            start=st, stop=sp)
                                    else:
                                        nc.tensor.matmul(
                                            ps_of[kk][:], lhsT=s4[:, ch % 8, :],
                                            rhs=gt[:, local, :],
                                            start=st, stop=sp)
                                ch += 1
                                local += 1
                    if "mm" not in skip:
                        for kk in ks:
                            chunk_sink(kk, ps_of[kk])
                assert ch == n_chunks

            def one_iter(hp_full, hs_full):
                # ------------- P1: hp = (feat @ W1) * ns -------------
                with tc.tile_pool(name="featT", bufs=1) as fpool, \
                     tc.tile_pool(name="p1work", bufs=4) as wpool:
                    fT = fpool.tile([P, kin, sh], BF16, tag="fT", name="fT")
                    STRIP = 8
                    for s0 in range(0, nsb, STRIP):
                        s1 = min(s0 + STRIP, nsb)
                        # load only this strip's feature columns so the first
                        # matmuls (and thus AG1) start ~30us earlier
                        nc.sync.dma_start(
                            out=fT[:, :, s0 * P:s1 * P],
                            in_=featT_d[:].rearrange(
                                "p (k s) -> p k s", k=kin)[:, :, s0 * P:s1 * P])
                        strip = wpool.tile([P, STRIP, OUT], BF16,
                                           tag="hpstrip", name="hpstrip")
                        for rt in range(s0, s1):
                            ps = psum.tile([P, OUT], F32, tag="acc",
                                           name="p1ps")
                            for kc in range(kin):
                                nc.tensor.matmul(
                                    ps[:],
                                    lhsT=fT[:, kc, rt * P:(rt + 1) * P],
                                    rhs=W1_t[:, kc, :],
                                    start=(kc == 0), stop=(kc == kin - 1))
                            nc.vector.tensor_scalar_mul(
                                strip[:, rt - s0, :], ps[:],
                                ns_t[:, rt:rt + 1])
                        nc.sync.dma_start(
                            out=hp_bounce[:].rearrange("(t p) o -> p t o",
                                                       p=P)[:, s0:s1, :],
                            in_=strip[:, :s1 - s0, :])

                if "ag" not in skip:
                    nc.gpsimd.collective_compute(
                        "AllGather", mybir.AluOpType.bypass,
                        ins=[hp_bounce.opt()], outs=[hp_full.opt()],
                        replica_groups=replica)

                # ------------- P2: gather+aggregate layer 1 -> hs ------
                with tc.tile_pool(name="g1", bufs=4) as gpool, \
                     tc.tile_pool(name="s1", bufs=4) as spool, \
                     tc.tile_pool(name="h1", bufs=4) as hpool, \
                     tc.tile_pool(name="hss", bufs=2) as hsspool:

                    hs_strips = {}

                    def sink1(kk, ps):
                        g = kk // SG
                        j = kk % SG
                        if j == 0:
                            hs_strips[g] = hsspool.tile(
                                [P, SG, P], BF16, tag="hss", name=f"hss_{g}")
                        t1 = hpool.tile([P, OUT], F32, tag="t1", name="t1")
                        nc.vector.tensor_scalar_mul(t1[:], ps[:],
                                                    nd_t[:, kk:kk + 1])
                        nc.vector.tensor_tensor(out=t1[:], in0=t1[:],
                                                in1=b1_t[:],
                                                op=mybir.AluOpType.add)
                        hrow = hpool.tile([P, OUT], F32, tag="hrow",
                                          name="hrow")
                        nc.scalar.activation(
                            hrow[:], t1[:],
                            mybir.ActivationFunctionType.Relu)
                        nc.vector.tensor_scalar_mul(hs_strips[g][:, j, :],
                                                    hrow[:],
                                                    ns_t[:, kk:kk + 1])
                        last = (kk == nsb - 1)
                        if j == SG - 1 or last:
                            n = j + 1
                            k0 = kk - j
                            nc.sync.dma_start(
                                out=hs_bounce[:].rearrange(
                                    "(t p) o -> p t o",
                                    p=P)[:, k0:k0 + n, :],
                                in_=hs_strips[g][:, :n, :])

                    if "gather" not in skip:
                        gather_pass([hp_full[bb * brows:(bb + 1) * brows, :]
                                     for bb in range(NBUCK)], gpool,
                                    spool, sink1, transposed=False)

                if "ag" not in skip:
                    nc.gpsimd.collective_compute(
                        "AllGather", mybir.AluOpType.bypass,
                        ins=[hs_bounce.opt()], outs=[hs_full.opt()],
                        replica_groups=replica)

                # ------------- P4: gather+aggregate layers 2/3 ----------
                with tc.tile_pool(name="g2", bufs=4) as gpool2, \
                     tc.tile_pool(name="s2", bufs=4) as spool2, \
                     tc.tile_pool(name="e2", bufs=6) as epool, \
                     tc.tile_pool(name="noisep", bufs=1) as npool, \
                     tc.tile_pool(name="outs", bufs=2) as outpool:

                    noise_t = npool.tile([P, nsb, OUT], F32, tag="noise",
                                         name="noise")
                    nc.sync.dma_start(out=noise_t[:],
                                      in_=noise_d[:].rearrange(
                                          "p (k o) -> p k o", k=nsb))

                    out_strips = {}

                    def sink2(kk, ps):
                        g = kk // SG
                        j = kk % SG
                        if j == 0:
                            out_strips[g] = outpool.tile(
                                [P, SG, OUT], F32, tag="outs",
                                name=f"os_{g}")
                        # ps is aggT [feat, dst] -> use directly as lhsT
                        aggs = epool.tile([P, OUT], BF16, tag="aggs",
                                          name="aggs")
                        nc.scalar.activation(
                            aggs[:], ps[:],
                            mybir.ActivationFunctionType.Copy)
                        ps23 = psum.tile([P, F2], F32, tag="acc",
                                         name="ps23")
                        nc.tensor.matmul(ps23[:], lhsT=aggs[:], rhs=W23_t[:],
                                         start=True, stop=True)
                        tmu = epool.tile([P, OUT], F32, tag="tmu", name="tmu")
                        nc.vector.tensor_scalar_mul(tmu[:], ps23[:, 0:OUT],
                                                    nd_t[:, kk:kk + 1])
                        nc.vector.tensor_tensor(out=tmu[:], in0=tmu[:],
                                                in1=bmu_t[:],
                                                op=mybir.AluOpType.add)
                        tls = epool.tile([P, OUT], F32, tag="tls", name="tls")
                        nc.vector.tensor_scalar_mul(tls[:], ps23[:, OUT:F2],
                                                    nd_t[:, kk:kk + 1])
                        nc.vector.tensor_tensor(out=tls[:], in0=tls[:],
                                                in1=bls_t[:],
                                                op=mybir.AluOpType.add)
                        sig = epool.tile([P, OUT], F32, tag="sig", name="sig")
                        nc.scalar.activation(
                            sig[:], tls[:],
                            mybir.ActivationFunctionType.Exp)
                        nc.vector.tensor_tensor(out=sig[:], in0=sig[:],
                                                in1=noise_t[:, kk, :],
                                                op=mybir.AluOpType.mult)
                        nc.vector.tensor_tensor(out=out_strips[g][:, j, :],
                                                in0=tmu[:], in1=sig[:],
                                                op=mybir.AluOpType.add)
                        last = (kk == nsb - 1)
                        if j == SG - 1 or last:
                            n = j + 1
                            k0 = kk - j
                            nc.sync.dma_start(
                                out=y_d[:].rearrange("(t p) o -> p t o",
                                                     p=P)[:, k0:k0 + n, :],
                                in_=out_strips[g][:, :n, :])

                    if "gather" not in skip:
                        gather_pass([hs_full[bb * brows:(bb + 1) * brows, :]
                                     for bb in range(NBUCK)], gpool2,
                                    spool2, sink2, transposed=True)

            for _rep in range(repeat):
                one_iter(hp_fulls[_rep], hs_fulls[_rep])

    nc.compile()
    return nc, in_maps, {"N": N, "perm": perm}


_CACHE = {}


def _run(feat, edges, W1, b1, W_mu, b_mu, W_ls, b_ls, noise, trace=False):
    import hashlib
    h = hashlib.sha1()
    for a in (edges, feat, W1, b1, W_mu, b_mu, W_ls, b_ls, noise):
        h.update(np.ascontiguousarray(a).tobytes())
    key = h.hexdigest()
    if key in _CACHE:
        nc, in_maps, meta = _CACHE[key]
    else:
        nc, in_maps, meta = _build(feat, edges, W1, b1, W_mu, b_mu, W_ls,
                                   b_ls, noise)
        _CACHE[key] = (nc, in_maps, meta)
    res = run_bass_kernel_spmd(nc, in_maps, core_ids=list(range(NC)),
                               trace=trace)
    out = np.concatenate([res.results[c]["y"] for c in range(NC)], axis=0)
    return out[meta["perm"][:meta["N"]]], res


def kernel(feat, edges, W1, b1, W_mu, b_mu, W_ls, b_ls, noise):
    out, _ = _run(np.asarray(feat), np.asarray(edges), np.asarray(W1),
                  np.asarray(b1), np.asarray(W_mu), np.asarray(b_mu),
                  np.asarray(W_ls), np.asarray(b_ls), np.asarray(noise))
    return out


# revision 6
# speedup vs baseline: 1.5072x; 1.2037x over previous
"""GraphConv VAE encoder (3x GraphConv + reparameterization) on 8 Trainium2 cores.

Strategy (graph/data parallel, dst-sharded, aggregate-then-project):
  - Nodes padded to NPAD = 8*SH and sharded by dst across 8 cores.
  - P1: hp = (feat @ W1) * ns computed per-core on its own node shard (host
    pre-transposes feat so no on-chip transposes are needed), then
    AllGather -> full bf16 gather table hp_full [npad, 128].
  - Edges are dst-sorted into 128-dst "sblocks", grouped 8 sblocks per
    supergroup, and split into 4 src-range buckets (dma_gather indices are
    int16, so gather tables are addressed in 4 windows of NPAD/4 rows).
  - P2: per 128-edge chunk, dma_gather source rows (partition = edge), build a
    one-hot selection matrix S via iota==dstloc on DVE, matmul S^T @ rows into
    a per-sblock PSUM accumulator (segment-sum on the tensor engine).  Sink:
    h = relu(agg*nd + b1); hs = h*ns stored ROW-major -> AllGather ->
    hs_full [npad, 128].
  - P4: same gather structure over hs_full, but accumulated TRANSPOSED:
    aggT = rows^T @ S ([feat, dst] in PSUM).  Since segment-sum commutes with
    the right-weight matmul, project only the local shard afterwards:
    ps23 = aggT^T @ [W_mu | W_ls] directly (aggT is already the lhsT).
    Epilogue mu + noise * exp(log_sigma) fused per sblock.
  - No replicated layer-2/3 projection over all nodes, no transposes.
  - The gather passes are descriptor-throughput-bound (~8.8ns/descriptor
    through the SWDGE path), so nodes are REBALANCED host-side: a
    quarter-preserving permutation assigns nodes to (core, sblock) so every
    (sblock, src-bucket) cell holds <= 512 edges = exactly 4 chunks of 128
    (npad is grown so cells average < 500, giving the balancer slack in
    both load and node count).  1953 -> 1600 chunks/core/pass.
  - Each core's shard is quartile-pure (sblocks [25q, 25q+25) hold only
    id-quarter-q nodes), so gather bucket q = concat of all cores' q-th
    quartile blocks, and each layer boundary's AllGather splits into 4
    slice-AGs that fire progressively as P1/P2 produce each quartile: the
    first P2 gather call only waits on piece 0 (~25% into P1) and P4's
    calls ramp while the last hs piece is still gathering, keeping the
    descriptor engine busy through both layer boundaries.
"""

import sys

sys.path.insert(0, '/opt/trn_rl_repo')

import numpy as np
import ml_dtypes

import concourse.bass as bass
import concourse.bacc as bacc
import concourse.mybir as mybir
import concourse.tile as tile
from concourse import library_config
from concourse.tile_rust import add_dep_helper
from concourse.vector_clock import ScopedClock
from concourse.bass_utils import run_bass_kernel_spmd

BF16 = mybir.dt.bfloat16
F32 = mybir.dt.float32
NPBF16 = ml_dtypes.bfloat16

NC = 8          # cores
P = 128         # partitions / sblock width
SG = 8          # sblocks per supergroup (one PSUM bank per sblock)
NBUCK = 4       # src-range buckets (int16 gather index limit)
PAD_DSTLOC = 256.0  # dstloc value for padded slots (never matches iota 0..127)


def _patch_tile_drain():
    """This walrus build rejects >1 sync-wait on the kernel-tail Drain; spread
    the waits across chained drains."""
    if getattr(tile.TileContext, "_drain_patched", False):
        return

    def patched(self, tick_clock, wait_clock):
        drain_inst = self.nc.sync.drain()
        wait_clock.add_sem_waits(drain_inst.ins,
                                 ScopedClock({None: tick_clock.global_clock}))
        si = drain_inst.ins.sync_info
        if si is not None and si.on_wait and len(si.on_wait) > 1:
            waits = list(si.on_wait)
            si.on_wait = waits[:1]
            for w in waits[1:]:
                d2 = self.nc.sync.drain()
                d2.ins.sync_info = mybir.SyncInfo(on_wait=[w], on_update=[])
        self.nc.all_engine_barrier()
        assert self.sems is not None
        popped = self.nc._tile_sem_poison_stack.pop()
        assert popped is self._sem_poison
        self.nc.clear_and_free_semaphores(list(self.sems.allocated().values()))
        self.nc.all_engine_barrier()

    tile.TileContext._drain_and_barrier = patched
    tile.TileContext._drain_patched = True


def _balance_perm(edges, n_nodes, npad):
    """Node -> table-row permutation equalizing per-(sblock, src-bucket)
    edge counts under 512 (= 4 chunks of 128).  Quarter-preserving for real
    nodes (so src buckets are stable); pad rows spread evenly as slack."""
    sh = npad // NC
    nsb = sh // P
    brows = npad // NBUCK
    qreal = n_nodes // NBUCK
    src = edges[0].astype(np.int64)
    dst = edges[1].astype(np.int64)
    hb = np.zeros((n_nodes, NBUCK), np.int32)
    np.add.at(hb, (dst, np.minimum(src // qreal, NBUCK - 1)), 1)

    def balance_quarter(hv, nbins):
        n = hv.shape[0]
        per = n // nbins
        deg = hv.sum(1)
        rate = hv.sum() / n / NBUCK
        loads = np.zeros((nbins, NBUCK), np.int32)
        counts = np.zeros(nbins, np.int32)
        assign = np.empty(n, np.int32)
        order = np.argsort(-deg, kind='stable')
        CAP = 4 * P
        for v in order:
            h = hv[v]
            newload = loads + h
            fits = (counts < per) & (newload <= CAP).all(1)
            if fits.any():
                resid = newload - (counts[:, None] + 1) * rate
                score = np.where(fits, resid.max(1), 1 << 30)
                s = int(np.argmin(score))
            else:
                over = np.maximum(newload - CAP, 0).sum(1)
                score = np.where(counts < per,
                                 over.astype(np.int64) * (1 << 20)
                                 + newload.max(1), 1 << 62)
                s = int(np.argmin(score))
            assign[v] = s
            loads[s] += h
            counts[s] += 1
        for _ in range(8):
            over_bins = np.where((loads > CAP).any(1))[0]
            if not len(over_bins):
                break
            moved = 0
            slack_order = np.argsort(loads.max(1))
            for s in over_bins:
                members_s = np.where(assign == s)[0]
                vs = members_s[np.argsort(-hv[members_s].sum(1))]
                done = False
                for v in vs[:32]:
                    hvv = hv[v]
                    cap_s = CAP - loads[s] + hvv
                    if (cap_s < 0).any():
                        continue
                    for t in slack_order:
                        if t == s:
                            continue
                        need_t = loads[t] + hvv - CAP
                        members_t = np.where(assign == t)[0]
                        hwt = hv[members_t]
                        ok = ((hwt <= cap_s).all(1)
                              & (hwt >= need_t).all(1))
                        if ok.any():
                            w = members_t[np.argmax(ok)]
                            loads[s] += hv[w] - hvv
                            loads[t] += hvv - hv[w]
                            assign[v], assign[w] = t, s
                            moved += 1
                            done = True
                            break
                    if done:
                        break
            if not moved:
                break
        return assign, loads

    # quartile-pure placement: quarter-q nodes go to sblocks
    # [q*nbq, (q+1)*nbq) of EVERY core, so gather bucket q = the contiguous
    # concatenation of all cores' q-th quartile blocks and each AllGather
    # can be split into 4 progressive slice-AGs.
    nbq = nsb // NBUCK
    perm_out = np.empty(npad, np.int64)     # node -> c*sh + kk*P + pos
    perm_tab = np.empty(npad, np.int64)     # node -> gather-table row
    prows = brows // NC                      # table rows per (core, piece)
    npadq = brows - qreal
    for q in range(NBUCK):
        real = np.arange(q * qreal, (q + 1) * qreal)
        hv = np.concatenate([hb[real],
                             np.zeros((npadq, NBUCK), np.int32)])
        nbins = NC * nbq
        assign, loads = balance_quarter(hv, nbins)
        chunkv = -(-loads // P)
        srt = {}
        for c in range(NC):
            bins = np.arange(c * nbq, (c + 1) * nbq)
            keys = [tuple(-chunkv[s2]) + (-loads[s2].sum(),)
                    for s2 in bins]
            order2 = sorted(range(nbq), key=lambda i: keys[i])
            for newk, i in enumerate(order2):
                srt[bins[i]] = newk
        orderv = np.argsort(assign, kind='stable')
        pos_in_bin = np.arange(len(hv)) - np.searchsorted(
            np.sort(assign), np.arange(nbins))[assign[orderv]]
        s_of = assign[orderv]
        c_of = s_of // nbq
        kloc = np.array([srt[s2] for s2 in s_of])
        ids = np.concatenate(
            [real, n_nodes + np.arange(q * npadq, (q + 1) * npadq)])
        perm_out[ids[orderv]] = (c_of * sh + (q * nbq + kloc) * P
                                 + pos_in_bin)
        perm_tab[ids[orderv]] = (q * brows + c_of * prows + kloc * P
                                 + pos_in_bin)
    assert np.array_equal(np.sort(perm_out), np.arange(npad))
    assert np.array_equal(np.sort(perm_tab), np.arange(npad))
    return perm_out, perm_tab


def _build_template(src_rows, dst_rows, npad):
    """Host-side edge preprocessing shared by both gather passes.

    src_rows: per-edge gather-TABLE row of the source (bucket-major);
    dst_rows: per-edge SHARD row of the destination (core-major).
    Returns the SPMD-shared template (chunk counts / call table / chunk
    metadata) and the per-core slot data (int16 gather indices, dstloc).
    """
    src = src_rows.astype(np.int64)
    dst = dst_rows.astype(np.int64)
    sh = npad // NC          # nodes per core shard
    nsb = sh // P            # sblocks per core
    brows = npad // NBUCK    # rows per gather bucket
    n_sg = (nsb + SG - 1) // SG
    sgs = [list(range(g * SG, min((g + 1) * SG, nsb))) for g in range(n_sg)]

    core = dst // sh
    k = (dst % sh) // P
    b = src // brows
    # cell id: (core, sg, b, k) major->minor defines the stream order
    sg_of_k = k // SG
    cell = ((core * n_sg + sg_of_k) * NBUCK + b) * nsb + k
    n_cells = NC * n_sg * NBUCK * nsb
    cnt = np.bincount(cell, minlength=n_cells).reshape(NC, n_sg, NBUCK, nsb)

    # shared chunk counts per (k, b): max over cores, >=1 chunk
    # C[k, b] indexed by absolute k
    C = np.zeros((nsb, NBUCK), np.int64)
    for g, ks in enumerate(sgs):
        for kk in ks:
            for bb in range(NBUCK):
                mx = cnt[:, g, bb, kk].max()
                C[kk, bb] = max(1, -(-int(mx) // P))

    # slot offsets in template order: for g: for b: for k in sgs[g]
    cell_order = []          # (g, b, k) in stream order
    for g, ks in enumerate(sgs):
        for bb in range(NBUCK):
            for kk in ks:
                cell_order.append((g, bb, kk))
    cell_slots = np.array([C[kk, bb] * P for (_, bb, kk) in cell_order])
    cell_off = np.concatenate([[0], np.cumsum(cell_slots)[:-1]])
    total_slots = int(cell_slots.sum())
    n_chunks = total_slots // P

    # call table: one dma_gather per (g, b)
    calls = []               # (g, b, slot_off, num_idxs)
    pos = 0
    for g, ks in enumerate(sgs):
        for bb in range(NBUCK):
            ni = int(sum(C[kk, bb] for kk in ks)) * P
            calls.append((g, bb, pos, ni))
            pos += ni
    assert pos == total_slots

    # chunk metadata in stream order: (k, g, b, start, stop)
    chunks = []
    for (g, bb, kk) in cell_order:
        nch = int(C[kk, bb])
        for j in range(nch):
            start = (bb == 0 and j == 0)
            stop = (bb == NBUCK - 1 and j == nch - 1)
            chunks.append((kk, g, bb, start, stop))
    assert len(chunks) == n_chunks

    # per-core slot data
    order = np.argsort(cell, kind='stable')
    cell_sorted = cell[order]
    # rank within cell
    cell_start = np.searchsorted(cell_sorted, np.arange(n_cells), side='left')
    rank = np.arange(len(order)) - cell_start[cell_sorted]
    # map cell -> slot offset (per its core's template)
    cell_to_off = np.zeros(n_cells, np.int64)
    for ci, (g, bb, kk) in enumerate(cell_order):
        for c in range(NC):
            gcell = ((c * n_sg + g) * NBUCK + bb) * nsb + kk
            cell_to_off[gcell] = cell_off[ci]
    slot = cell_to_off[cell_sorted] + rank

    idx_vals = np.zeros((NC, total_slots), np.int16)
    dl_vals = np.full((NC, total_slots), PAD_DSTLOC, np.float32)
    csrc = src[order] - b[order] * brows
    cdst = dst[order] % P
    ccore = core[order]
    idx_vals[ccore, slot] = csrc.astype(np.int16)
    dl_vals[ccore, slot] = cdst.astype(np.float32)

    # wrap indices per call: within a call, slot j -> [j%16, off//16 + j//16]
    ni16 = total_slots // 16
    idx16 = np.zeros((NC, 16, ni16), np.int16)
    for (_, _, off, ni) in calls:
        blk = idx_vals[:, off:off + ni].reshape(NC, ni // 16, 16)
        idx16[:, :, off // 16:(off + ni) // 16] = blk.transpose(0, 2, 1)
    idx16 = np.tile(idx16, (1, 8, 1))  # replicate to 128 partitions

    # dstloc per chunk column: [p, ch] = dstloc of slot ch*128+p
    dstloc = dl_vals.reshape(NC, n_chunks, P).transpose(0, 2, 1)  # [NC,128,NCH]
    dstloc = dstloc.astype(NPBF16)

    tpl = dict(sh=sh, nsb=nsb, brows=brows, sgs=sgs, calls=calls,
               chunks=chunks, n_chunks=n_chunks, total_slots=total_slots,
               ni16=ni16)
    return tpl, idx16, dstloc


def _build(feat, edges, W1, b1, W_mu, b_mu, W_ls, b_ls, noise):
    import os
    skip = os.environ.get("K_SKIP", "")
    repeat = int(os.environ.get("K_REPEAT", "1"))
    single_packet = bool(int(os.environ.get("K_SP", "0")))
    N, IN = feat.shape
    E = edges.shape[1]
    OUT = W1.shape[1]
    F2 = 2 * OUT
    assert OUT == P
    npad = -(-N // (NC * P)) * NC * P        # multiple of 8*128
    # bucket rows must fit int16 and divide into 128-aligned shards
    while npad % (NBUCK * P) != 0:
        npad += NC * P
    # extra padding so balanced cells average under 500 edges (<512 = 4
    # chunks), giving the balancer slack in both load and node count
    while E / NC / ((npad // NC // P) * NBUCK) > 500.0:
        npad += NC * P
    sh = npad // NC
    brows = npad // NBUCK
    assert brows <= 32768
    nsb = sh // P
    kin = IN // P

    while (npad // NC // P) % NBUCK != 0:
        npad += NC * P
        sh = npad // NC
        brows = npad // NBUCK
        nsb = sh // P
    nbq = nsb // NBUCK                       # sblocks per quartile piece

    perm_out, perm_tab = _balance_perm(edges, N, npad)
    inv = np.empty(npad, np.int64)
    inv[perm_out] = np.arange(npad)
    mask = inv < N                           # real rows
    e64 = edges.astype(np.int64)

    tpl, idx16, dstloc = _build_template(perm_tab[e64[0]],
                                         perm_out[e64[1]], npad)
    sgs, calls, chunks = tpl['sgs'], tpl['calls'], tpl['chunks']
    n_chunks, ni16 = tpl['n_chunks'], tpl['ni16']

    # ---- host-side numeric prep (degrees on node ids, then permuted) ----
    deg_out = np.bincount(edges[0], minlength=N).astype(np.float64)
    deg_in = np.bincount(edges[1], minlength=N).astype(np.float64)
    ns_n = (np.clip(deg_out, 1.0, None) ** -0.5).astype(np.float32)
    nd_n = (np.clip(deg_in, 1.0, None) ** -0.5).astype(np.float32)
    safe = np.minimum(inv, N - 1)
    ns = np.where(mask, ns_n[safe], 0.0).astype(np.float32)
    nd = np.where(mask, nd_n[safe], 0.0).astype(np.float32)

    featp = np.zeros((npad, IN), np.float32)
    featp[mask] = feat[inv[mask]]
    noisep = np.zeros((npad, OUT), np.float32)
    noisep[mask] = noise[inv[mask]]

    featb = featp.astype(NPBF16)
    W1b = np.ascontiguousarray(W1.astype(NPBF16))
    W23 = np.concatenate([W_mu, W_ls], axis=1)
    W23b = np.ascontiguousarray(W23.astype(NPBF16))
    # W1 as [128, kin, 128]: [p, kc, j] = W1[kc*128+p, j]
    W1sb = np.ascontiguousarray(W1b.reshape(kin, P, OUT).transpose(1, 0, 2))

    iota4 = np.tile(np.arange(P, dtype=np.float32), 8)[None, :].repeat(P, 0)
    iota4 = iota4.astype(NPBF16)                       # [128, 1024]
    b1r = np.tile(b1[None, :].astype(np.float32), (P, 1))
    bmur = np.tile(b_mu[None, :].astype(np.float32), (P, 1))
    blsr = np.tile(b_ls[None, :].astype(np.float32), (P, 1))

    in_maps = []
    for c in range(NC):
        rows = slice(c * sh, (c + 1) * sh)
        fsh = featb[rows]                               # [sh, IN]
        featT = np.ascontiguousarray(
            fsh.T.reshape(kin, P, sh).transpose(1, 0, 2).reshape(P, kin * sh))
        nsc = np.ascontiguousarray(
            ns[rows].reshape(nsb, P).T)                 # [128, nsb]
        ndc = np.ascontiguousarray(nd[rows].reshape(nsb, P).T)
        noc = np.ascontiguousarray(
            noisep[rows].reshape(nsb, P, OUT).transpose(1, 0, 2)
            .reshape(P, nsb * OUT))                     # [128, nsb*128]
        in_maps.append({
            "featT": featT, "W1sb": W1sb.reshape(P, kin * OUT),
            "W23sb": W23b, "b1r": b1r, "bmur": bmur, "blsr": blsr,
            "nsc": nsc, "ndc": ndc, "noise_sb": noc,
            "iota4": iota4,
            "idx16": np.ascontiguousarray(idx16[c]),
            "dstloc": np.ascontiguousarray(dstloc[c]),
        })

    # ---------------- device program ----------------
    _patch_tile_drain()
    nc = bacc.Bacc('TRN2', target_bir_lowering=False, debug=False)

    featT_d = nc.dram_tensor("featT", [P, kin * sh], BF16, kind="ExternalInput")
    W1_d = nc.dram_tensor("W1sb", [P, kin * OUT], BF16, kind="ExternalInput")
    W23_d = nc.dram_tensor("W23sb", [P, F2], BF16, kind="ExternalInput")
    b1_d = nc.dram_tensor("b1r", [P, OUT], F32, kind="ExternalInput")
    bmu_d = nc.dram_tensor("bmur", [P, OUT], F32, kind="ExternalInput")
    bls_d = nc.dram_tensor("blsr", [P, OUT], F32, kind="ExternalInput")
    ns_d = nc.dram_tensor("nsc", [P, nsb], F32, kind="ExternalInput")
    nd_d = nc.dram_tensor("ndc", [P, nsb], F32, kind="ExternalInput")
    noise_d = nc.dram_tensor("noise_sb", [P, nsb * OUT], F32,
                             kind="ExternalInput")
    iota_d = nc.dram_tensor("iota4", [P, 8 * P], BF16, kind="ExternalInput")
    idx_d = nc.dram_tensor("idx16", [P, ni16], mybir.dt.int16,
                           kind="ExternalInput")
    dl_d = nc.dram_tensor("dstloc", [P, n_chunks], BF16, kind="ExternalInput")
    y_d = nc.dram_tensor("y", [sh, OUT], F32, kind="ExternalOutput")

    replica = [list(range(NC))]

    with tile.TileContext(nc) as tc:
        import contextlib
        with contextlib.ExitStack() as ctx:
            dram = ctx.enter_context(tc.tile_pool(name="dram", bufs=1,
                                                  space="DRAM"))
            cpool = ctx.enter_context(tc.tile_pool(name="const", bufs=1))
            psum = ctx.enter_context(tc.tile_pool(name="psum", bufs=SG,
                                                  space="PSUM"))

            hp_bounce_p = [dram.tile([nbq * P, OUT], BF16,
                                     tag=f"hp_bnc_{j}", name=f"hp_bnc_{j}")
                           for j in range(NBUCK)]
            hs_bounce_p = [dram.tile([nbq * P, OUT], BF16,
                                     tag=f"hs_bnc_{j}", name=f"hs_bnc_{j}")
                           for j in range(NBUCK)]
            hp_fulls = [[dram.tile([brows, OUT], BF16,
                                   tag=f"hp_full_{r}_{j}",
                                   addr_space="Shared",
                                   name=f"hp_full_{r}_{j}")
                         for j in range(NBUCK)] for r in range(repeat)]
            hs_fulls = [[dram.tile([brows, OUT], BF16,
                                   tag=f"hs_full_{r}_{j}",
                                   addr_space="Shared",
                                   name=f"hs_full_{r}_{j}")
                         for j in range(NBUCK)] for r in range(repeat)]

            # constants
            W1_t = cpool.tile([P, kin, OUT], BF16, tag="w1")
            W23_t = cpool.tile([P, F2], BF16, tag="w23")
            b1_t = cpool.tile([P, OUT], F32, tag="b1")
            bmu_t = cpool.tile([P, OUT], F32, tag="bmu")
            bls_t = cpool.tile([P, OUT], F32, tag="bls")
            ns_t = cpool.tile([P, nsb], F32, tag="ns")
            nd_t = cpool.tile([P, nsb], F32, tag="nd")
            iota_t = cpool.tile([P, 8, P], BF16, tag="iota")
            idx_t = cpool.tile([P, ni16], mybir.dt.int16, tag="idx")
            dl_t = cpool.tile([P, n_chunks], BF16, tag="dl")
            nc.sync.dma_start(out=W1_t[:], in_=W1_d[:].rearrange(
                "p (k o) -> p k o", k=kin))
            nc.sync.dma_start(out=W23_t[:], in_=W23_d[:])
            nc.sync.dma_start(out=b1_t[:], in_=b1_d[:])
            nc.sync.dma_start(out=bmu_t[:], in_=bmu_d[:])
            nc.sync.dma_start(out=bls_t[:], in_=bls_d[:])
            nc.sync.dma_start(out=ns_t[:], in_=ns_d[:])
            nc.sync.dma_start(out=nd_t[:], in_=nd_d[:])
            nc.sync.dma_start(out=iota_t[:], in_=iota_d[:].rearrange(
                "p (a b) -> p a b", a=8))
            nc.sync.dma_start(out=idx_t[:], in_=idx_d[:])
            nc.sync.dma_start(out=dl_t[:], in_=dl_d[:])

            reload_inst = nc.gpsimd.load_library(library_config.mlp)

            _cpk = {}
            for (k2, g2, b2, st2, sp2) in chunks:
                _cpk[(k2, b2)] = _cpk.get((k2, b2), 0) + 1

            max_call_chunks = max(ni // P for (_, _, _, ni) in calls)

            def gather_pass(table_aps, gpool, spool, chunk_sink,
                            transposed):
                """Per-(sg,b) dma_gather calls, S build per 8 chunks, matmul
                per chunk into per-sblock psums.  transposed=False:
                ps[dst, feat] += S^T @ rows; transposed=True:
                ps[feat, dst] += rows^T @ S.  chunk_sink(k_abs, ps) is called
                when a sblock finishes."""
                ps_of = {}
                s4 = None
                ch = 0
                ci = 0
                for g, ks in enumerate(sgs):
                    for kk in ks:
                        ps_of[kk] = psum.tile([P, OUT], F32, tag="acc",
                                              name=f"acc_{kk}")
                    for bb in range(NBUCK):
                        (gg, bb2, off, nidx) = calls[ci]
                        assert gg == g and bb2 == bb
                        ci += 1
                        gt = gpool.tile([P, max_call_chunks, OUT], BF16,
                                        tag="gt")
                        if not ("g23" in skip and bb >= 2):
                            gi = nc.gpsimd.dma_gather(
                                out_ap=gt[:, :nidx // P, :],
                                in_ap=table_aps[bb],
                                idxs_ap=idx_t[:, off // 16:(off + nidx) // 16],
                                num_idxs=nidx, num_idxs_reg=nidx,
                                elem_size=OUT, single_packet=single_packet)
                            add_dep_helper(gi.ins, reload_inst.ins,
                                           sync=False)
                        local = 0
                        for kk in ks:
                            nchk = _cpk[(kk, bb)]
                            for j in range(nchk):
                                if ch % 8 == 0 and "sb" not in skip:
                                    s4 = spool.tile([P, 8, P], BF16, tag="s4")
                                    n4 = min(8, n_chunks - ch)
                                    nc.vector.tensor_tensor(
                                        out=s4[:, :n4, :],
                                        in0=iota_t[:, :n4, :],
                                        in1=dl_t[:, ch:ch + n4, None]
                                        .to_broadcast([P, n4, P]),
                                        op=mybir.AluOpType.is_equal)
                                elif s4 is None:
                                    s4 = spool.tile([P, 8, P], BF16, tag="s4")
                                kk_, g_, bb_, st, sp = chunks[ch]
                                assert kk_ == kk and g_ == g and bb_ == bb
                                if "mm" not in skip:
                                    if transposed:
                                        nc.tensor.matmul(
                                            ps_of[kk][:], lhsT=gt[:, local, :],
                                            rhs=s4[:, ch % 8, :],
                                            start=st, stop=sp)
                                    else:
                                        nc.tensor.matmul(
                                            ps_of[kk][:], lhsT=s4[:, ch % 8, :],
                                            rhs=gt[:, local, :],
                                            start=st, stop=sp)
                                ch += 1
                                local += 1
                    if "mm" not in skip:
                        for kk in ks:
                            chunk_sink(kk, ps_of[kk])
                assert ch == n_chunks

            def one_iter(hp_full, hs_full):
                # ------------- P1: hp = (feat @ W1) * ns -------------
                # strips are quartile-aligned (STRIP divides nbq); each
                # quartile piece AllGathers as soon as its strips land, so
                # P2's first gather calls start ~140us earlier
                with tc.tile_pool(name="featT", bufs=1) as fpool, \
                     tc.tile_pool(name="p1work", bufs=4) as wpool:
                    fT = fpool.tile([P, kin, sh], BF16, tag="fT", name="fT")
                    STRIP = 5
                    assert nbq % STRIP == 0
                    for s0 in range(0, nsb, STRIP):
                        s1 = min(s0 + STRIP, nsb)
                        pj = s0 // nbq
                        nc.sync.dma_start(
                            out=fT[:, :, s0 * P:s1 * P],
                            in_=featT_d[:].rearrange(
                                "p (k s) -> p k s", k=kin)[:, :, s0 * P:s1 * P])
                        strip = wpool.tile([P, STRIP, OUT], BF16,
                                           tag="hpstrip", name="hpstrip")
                        for rt in range(s0, s1):
                            ps = psum.tile([P, OUT], F32, tag="acc",
                                           name="p1ps")
                            for kc in range(kin):
                                nc.tensor.matmul(
                                    ps[:],
                                    lhsT=fT[:, kc, rt * P:(rt + 1) * P],
                                    rhs=W1_t[:, kc, :],
                                    start=(kc == 0), stop=(kc == kin - 1))
                            nc.vector.tensor_scalar_mul(
                                strip[:, rt - s0, :], ps[:],
                                ns_t[:, rt:rt + 1])
                        nc.sync.dma_start(
                            out=hp_bounce_p[pj][:].rearrange(
                                "(t p) o -> p t o",
                                p=P)[:, s0 - pj * nbq:s1 - pj * nbq, :],
                            in_=strip[:, :s1 - s0, :])
                        if s1 % nbq == 0 and "ag" not in skip:
                            nc.gpsimd.collective_compute(
                                "AllGather", mybir.AluOpType.bypass,
                                ins=[hp_bounce_p[pj].opt()],
                                outs=[hp_full[pj].opt()],
                                replica_groups=replica)

                # ------------- P2: gather+aggregate layer 1 -> hs ------
                with tc.tile_pool(name="g1", bufs=4) as gpool, \
                     tc.tile_pool(name="s1", bufs=4) as spool, \
                     tc.tile_pool(name="h1", bufs=4) as hpool, \
                     tc.tile_pool(name="hss", bufs=2) as hsspool:

                    hs_strips = {}

                    def sink1(kk, ps):
                        g = kk // SG
                        j = kk % SG
                        if j == 0:
                            hs_strips[g] = hsspool.tile(
                                [P, SG, P], BF16, tag="hss", name=f"hss_{g}")
                        t1 = hpool.tile([P, OUT], F32, tag="t1", name="t1")
                        nc.vector.tensor_scalar_mul(t1[:], ps[:],
                                                    nd_t[:, kk:kk + 1])
                        nc.vector.tensor_tensor(out=t1[:], in0=t1[:],
                                                in1=b1_t[:],
                                                op=mybir.AluOpType.add)
                        hrow = hpool.tile([P, OUT], F32, tag="hrow",
                                          name="hrow")
                        nc.scalar.activation(
                            hrow[:], t1[:],
                            mybir.ActivationFunctionType.Relu)
                        nc.vector.tensor_scalar_mul(hs_strips[g][:, j, :],
                                                    hrow[:],
                                                    ns_t[:, kk:kk + 1])
                        last = (kk == nsb - 1)
                        if j == SG - 1 or last:
                            n = j + 1
                            k0 = kk - j
                            # split the strip write at quartile-piece
                            # boundaries; AllGather each piece when its
                            # last sblock lands
                            a = k0
                            while a < k0 + n:
                                pj = a // nbq
                                b2 = min(k0 + n, (pj + 1) * nbq)
                                nc.sync.dma_start(
                                    out=hs_bounce_p[pj][:].rearrange(
                                        "(t p) o -> p t o",
                                        p=P)[:, a - pj * nbq:b2 - pj * nbq,
                                             :],
                                    in_=hs_strips[g][:, a - k0:b2 - k0, :])
                                if (b2 % nbq == 0 or b2 == nsb) \
                                        and "ag" not in skip:
                                    nc.gpsimd.collective_compute(
                                        "AllGather",
                                        mybir.AluOpType.bypass,
                                        ins=[hs_bounce_p[pj].opt()],
                                        outs=[hs_full[pj].opt()],
                                        replica_groups=replica)
                                a = b2

                    if "gather" not in skip:
                        gather_pass([hp_full[bb][:]
                                     for bb in range(NBUCK)], gpool,
                                    spool, sink1, transposed=False)

                # ------------- P4: gather+aggregate layers 2/3 ----------
                with tc.tile_pool(name="g2", bufs=4) as gpool2, \
                     tc.tile_pool(name="s2", bufs=4) as spool2, \
                     tc.tile_pool(name="e2", bufs=6) as epool, \
                     tc.tile_pool(name="noisep", bufs=1) as npool, \
                     tc.tile_pool(name="outs", bufs=2) as outpool:

                    noise_t = npool.tile([P, nsb, OUT], F32, tag="noise",
                                         name="noise")
                    nc.sync.dma_start(out=noise_t[:],
                                      in_=noise_d[:].rearrange(
                                          "p (k o) -> p k o", k=nsb))

                    out_strips = {}

                    def sink2(kk, ps):
                        g = kk // SG
                        j = kk % SG
                        if j == 0:
                            out_strips[g] = outpool.tile(
                                [P, SG, OUT], F32, tag="outs",
                                name=f"os_{g}")
                        # ps is aggT [feat, dst] -> use directly as lhsT
                        aggs = epool.tile([P, OUT], BF16, tag="aggs",
                                          name="aggs")
                        nc.scalar.activation(
                            aggs[:], ps[:],
                            mybir.ActivationFunctionType.Copy)
                        ps23 = psum.tile([P, F2], F32, tag="acc",
                                         name="ps23")
                        nc.tensor.matmul(ps23[:], lhsT=aggs[:], rhs=W23_t[:],
                                         start=True, stop=True)
                        tmu = epool.tile([P, OUT], F32, tag="tmu", name="tmu")
                        nc.vector.tensor_scalar_mul(tmu[:], ps23[:, 0:OUT],
                                                    nd_t[:, kk:kk + 1])
                        nc.vector.tensor_tensor(out=tmu[:], in0=tmu[:],
                                                in1=bmu_t[:],
                                                op=mybir.AluOpType.add)
                        tls = epool.tile([P, OUT], F32, tag="tls", name="tls")
                        nc.vector.tensor_scalar_mul(tls[:], ps23[:, OUT:F2],
                                                    nd_t[:, kk:kk + 1])
                        nc.vector.tensor_tensor(out=tls[:], in0=tls[:],
                                                in1=bls_t[:],
                                                op=mybir.AluOpType.add)
                        sig = epool.tile([P, OUT], F32, tag="sig", name="sig")
                        nc.scalar.activation(
                            sig[:], tls[:],
                            mybir.ActivationFunctionType.Exp)
                        nc.vector.tensor_tensor(out=sig[:], in0=sig[:],
                                                in1=noise_t[:, kk, :],
                                                op=mybir.AluOpType.mult)
                        nc.vector.tensor_tensor(out=out_strips[g][:, j, :],
                                                in0=tmu[:], in1=sig[:],
                                                op=mybir.AluOpType.add)
                        last = (kk == nsb - 1)
                        if j == SG - 1 or last:
                            n = j + 1
                            k0 = kk - j
                            nc.sync.dma_start(
                                out=y_d[:].rearrange("(t p) o -> p t o",
                                                     p=P)[:, k0:k0 + n, :],
                                in_=out_strips[g][:, :n, :])

                    if "gather" not in skip:
                        gather_pass([hs_full[bb][:]
                                     for bb in range(NBUCK)], gpool2,
                                    spool2, sink2, transposed=True)

            for _rep in range(repeat):
                one_iter(hp_fulls[_rep], hs_fulls[_rep])

    nc.compile()
    return nc, in_maps, {"N": N, "perm": perm_out}


_CACHE = {}


def _run(feat, edges, W1, b1, W_mu, b_mu, W_ls, b_ls, noise, trace=False):
    import hashlib
    h = hashlib.sha1()
    for a in (edges, feat, W1, b1, W_mu, b_mu, W_ls, b_ls, noise):
        h.update(np.ascontiguousarray(a).tobytes())
    key = h.hexdigest()
    if key in _CACHE:
        nc, in_maps, meta = _CACHE[key]
    else:
        nc, in_maps, meta = _build(feat, edges, W1, b1, W_mu, b_mu, W_ls,
                                   b_ls, noise)
        _CACHE[key] = (nc, in_maps, meta)
    res = run_bass_kernel_spmd(nc, in_maps, core_ids=list(range(NC)),
                               trace=trace)
    out = np.concatenate([res.results[c]["y"] for c in range(NC)], axis=0)
    return out[meta["perm"][:meta["N"]]], res


def kernel(feat, edges, W1, b1, W_mu, b_mu, W_ls, b_ls, noise):
    out, _ = _run(np.asarray(feat), np.asarray(edges), np.asarray(W1),
                  np.asarray(b1), np.asarray(W_mu), np.asarray(b_mu),
                  np.asarray(W_ls), np.asarray(b_ls), np.asarray(noise))
    return out
